# revision 3
# baseline (speedup 1.0000x reference)
"""LoRA basis-bank kernel for 8 TRN2 NeuronCores.

Math (per batch b):
    A_mixed  = sum_k alpha[b,k] * A_bank[k]        # [R, DIN]
    B_mixedT = sum_k alpha[b,k] * B_bank[k].T      # [R, DOUT]
    z        = h[b] @ A_mixed.T                    # [S, R]
    delta[b] = z @ B_mixedT                        # [S, DOUT]

Sharding: data-parallel over batch, 1 batch per core; banks replicated.

Host-side layout prep (no arithmetic): h shard is uploaded transposed
(hT[i, s]) in bf16 so the DIN contraction lands on SBUF partitions with
no on-device transposes; mix/A_bank/B_bank.T are concatenated into one
[K*R, R + DIN + DOUT] bf16 "banks" tensor so they load in 2 big DMAs;
alpha is expanded into a [K*R, R] block-diagonal placement matrix.

Device dataflow per core (memory-roofline streaming schedule):
  - sync ring queue order: banks (2x ~1MB), hT chunks (16x 512KB).
  - A_mixT chunks [128i, R] = A_flat_chunk.T @ M as banks land
  - B_mixedT [R, DOUT] = M.T @ B_flat
  - mm1 runs c-outer / s-chunk-inner: each arriving hT chunk [128, S]
    is immediately consumed into 4 persistent PSUM accumulators
    zT[sc] [R, 512], so compute streams behind the loads.
  - mm2: per s-tile [128, DOUT] = zT.T @ B_mixedT, drains to bf16 SBUF
    via vector+scalar copies, stores on the scalar ring (16x 512KB).
  - delta written bf16, upcast to fp32 on host
"""

import ml_dtypes
import numpy as np

import concourse.bacc as bacc
import concourse.bass as bass
import concourse.mybir as mybir
import concourse.tile as tile
from concourse.bass_utils import run_bass_kernel_spmd

B, S, K, R, DIN, DOUT = 8, 2048, 16, 16, 2048, 2048
KR = K * R  # 256
BANKW = R + DIN + DOUT  # 16 + 2048 + 2048 = 4112
F32 = mybir.dt.float32
BF16 = mybir.dt.bfloat16

_cache = {}


def _build_nc():
    nc = bacc.Bacc("TRN2", target_bir_lowering=False)

    ht_d = nc.dram_tensor("hbT", [DIN, S], BF16, kind="ExternalInput")
    bank_d = nc.dram_tensor("banks", [KR, BANKW], BF16, kind="ExternalInput")
    out_d = nc.dram_tensor("delta", [S, DOUT], BF16, kind="ExternalOutput")

    NCH = DIN // 128  # 16 chunks along DIN
    NSC = S // 512    # 4 s-chunks
    with tile.TileContext(nc) as tc:
        with (
            tc.tile_pool(name="const", bufs=1) as constp,
            tc.tile_pool(name="hT", bufs=1) as hTp,
            tc.tile_pool(name="dout", bufs=3) as dp,
            tc.tile_pool(name="pssm", bufs=2, space="PSUM") as psmall,
            tc.tile_pool(name="psz", bufs=1, space="PSUM") as pszp,
            tc.tile_pool(name="psd", bufs=2, space="PSUM") as psdp,
        ):
            # ---- queue every load on the sync ring, in issue order ----
            bank_sb = []
            for half in range(2):
                bk = constp.tile([128, BANKW], BF16, tag=f"bank{half}")
                nc.sync.dma_start(
                    bk[:], bank_d[half * 128:(half + 1) * 128, :])
                bank_sb.append(bk)
            hTs = []
            for c in range(NCH):
                hT = hTp.tile([128, S], BF16, tag=f"hT{c}")
                nc.sync.dma_start(hT[:], ht_d[c * 128:(c + 1) * 128, :])
                hTs.append(hT)

            m_sb = [bk[:, 0:R] for bk in bank_sb]
            a_sb = [bk[:, R:R + DIN] for bk in bank_sb]
            b_sb = [bk[:, R + DIN:BANKW] for bk in bank_sb]

            # ---- A_mixT chunks direct: [128, R] = A_flat_chunk.T @ M ----
            amixT = []
            for c in range(NCH):
                csl = slice(c * 128, (c + 1) * 128)
                pat = psmall.tile([128, R], F32, tag="sm")
                nc.tensor.matmul(pat[:], a_sb[0][:, csl], m_sb[0],
                                 start=True, stop=False)
                nc.tensor.matmul(pat[:], a_sb[1][:, csl], m_sb[1],
                                 start=False, stop=True)
                t_sb = constp.tile([128, R], BF16, tag=f"amixT{c}")
                nc.vector.tensor_copy(t_sb[:], pat[:])
                amixT.append(t_sb)

            # ---- B_mixedT [R, DOUT] = M.T @ B_flat (to bf16) ----
            bmixT = constp.tile([R, DOUT], BF16, tag="bmixT")
            for c4 in range(DOUT // 512):
                sl = slice(c4 * 512, (c4 + 1) * 512)
                pmix = psmall.tile([R, 512], F32, tag="sm")
                nc.tensor.matmul(pmix[:], m_sb[0], b_sb[0][:, sl],
                                 start=True, stop=False)
                nc.tensor.matmul(pmix[:], m_sb[1], b_sb[1][:, sl],
                                 start=False, stop=True)
                nc.vector.tensor_copy(bmixT[:, sl], pmix[:])

            # ---- mm1, c-outer: consume each hT chunk as it arrives ----
            zt_ps = [pszp.tile([R, 512], F32, tag=f"zt{sc}",
                               name=f"zt_ps{sc}")
                     for sc in range(NSC)]
            for c in range(NCH):
                for sc in range(NSC):
                    nc.tensor.matmul(
                        zt_ps[sc][:], amixT[c][:],
                        hTs[c][:, sc * 512:(sc + 1) * 512],
                        start=(c == 0), stop=(c == NCH - 1))
            zts = []
            for sc in range(NSC):
                zt = constp.tile([R, 512], BF16, tag=f"z{sc}")
                nc.vector.tensor_copy(zt[:], zt_ps[sc][:])
                zts.append(zt)

            # ---- mm2 + stores: delta tile [128, DOUT] per s-tile ----
            for sc in range(NSC):
                for t in range(4):
                    row0 = (sc * 4 + t) * 128
                    dsb = dp.tile([128, DOUT], BF16, tag="d")
                    for oc in range(DOUT // 512):
                        osl = slice(oc * 512, (oc + 1) * 512)
                        dps = psdp.tile([128, 512], F32, tag="dps")
                        nc.tensor.matmul(
                            dps[:], zts[sc][:, t * 128:(t + 1) * 128],
                            bmixT[:, osl])
                        if oc % 2 == 0:
                            nc.vector.tensor_copy(dsb[:, osl], dps[:])
                        else:
                            nc.scalar.copy(dsb[:, osl], dps[:])
                    nc.scalar.dma_start(out_d[row0:row0 + 128, :], dsb[:])

    nc.compile()
    return nc


def _in_maps(h, alpha, A_bank, B_bank):
    a_flat = A_bank.reshape(KR, DIN).astype(np.float32)
    bt_flat = B_bank.transpose(0, 2, 1).reshape(KR, DOUT).astype(np.float32)
    eye = np.eye(R, dtype=np.float32)
    maps = []
    for b in range(B):
        mix = np.kron(alpha[b].astype(np.float32).reshape(K, 1), eye)
        banks = np.concatenate([mix, a_flat, bt_flat], axis=1)
        hT = np.ascontiguousarray(
            np.asarray(h[b]).T).astype(ml_dtypes.bfloat16)
        maps.append({
            "hbT": hT,
            "banks": np.ascontiguousarray(banks.astype(ml_dtypes.bfloat16)),
        })
    return maps


def _run(inputs, trace=False):
    if "nc" not in _cache:
        _cache["nc"] = _build_nc()
    nc = _cache["nc"]
    maps = _in_maps(inputs["h"], inputs["alpha"], inputs["A_bank"],
                    inputs["B_bank"])
    res = run_bass_kernel_spmd(nc, maps, core_ids=list(range(B)), trace=trace)
    out = np.stack([res.results[b]["delta"] for b in range(B)], axis=0)
    return out.astype(np.float32), res


def kernel(**inputs):
    out, _ = _run(inputs, trace=False)
    return out


# revision 5
# speedup vs baseline: 1.2192x; 1.2192x over previous
"""LoRA basis-bank kernel for 8 TRN2 NeuronCores.

Math (per batch b):
    A_mixed  = sum_k alpha[b,k] * A_bank[k]        # [R, DIN]
    B_mixedT = sum_k alpha[b,k] * B_bank[k].T      # [R, DOUT]
    z        = h[b] @ A_mixed.T                    # [S, R]
    delta[b] = z @ B_mixedT                        # [S, DOUT]

Sharding: data-parallel over batch, 1 batch per core; banks replicated.

Host-side layout prep (no arithmetic): h shard is uploaded transposed
(hT[i, s]) in bf16 so the DIN contraction lands on SBUF partitions with
no on-device transposes; mix/A_bank/B_bank.T are concatenated into one
[K*R, R + DIN + DOUT] bf16 "banks" tensor so they load in 2 big DMAs;
alpha is expanded into a [K*R, R] block-diagonal placement matrix.

Device dataflow per core (memory-roofline streaming schedule):
  - sync ring queue order: banks (2x ~1MB), hT chunks (16x 512KB),
    then the 16 delta stores (ring idle by then; scalar/vector engines
    stay free for PSUM drains).
  - A_mixT [128c, 16] chunks all land in ONE PSUM bank [128, 256],
    two casts to SBUF.
  - B_mixedT [R, DOUT] = M.T @ B_flat via the mm2 PSUM pool.
  - mm1 runs c-outer: each arriving hT chunk [128, S] is immediately
    consumed into one persistent PSUM tile zT [16, 2048] (4 banks),
    so compute streams behind the loads.
  - mm2: per s-tile [128, DOUT] = zT.T @ B_mixedT, 4 matmuls through
    a 3-deep PSUM pool, drained by alternating vector/scalar copies.
  - delta written bf16, upcast to fp32 on host
"""

import ml_dtypes
import numpy as np

import concourse.bacc as bacc
import concourse.bass as bass
import concourse.mybir as mybir
import concourse.tile as tile
from concourse.bass_utils import run_bass_kernel_spmd

B, S, K, R, DIN, DOUT = 8, 2048, 16, 16, 2048, 2048
KR = K * R  # 256
BANKW = R + DIN + DOUT  # 16 + 2048 + 2048 = 4112
F32 = mybir.dt.float32
BF16 = mybir.dt.bfloat16

_cache = {}


def _build_nc():
    nc = bacc.Bacc("TRN2", target_bir_lowering=False)

    ht_d = nc.dram_tensor("hbT", [DIN, S], BF16, kind="ExternalInput")
    bank_d = nc.dram_tensor("banks", [KR, BANKW], BF16, kind="ExternalInput")
    out_d = nc.dram_tensor("delta", [S, DOUT], BF16, kind="ExternalOutput")

    NCH = DIN // 128  # 16 chunks along DIN
    NSC = S // 512    # 4 s-chunks
    with tile.TileContext(nc) as tc:
        with (
            tc.tile_pool(name="const", bufs=1) as constp,
            tc.tile_pool(name="hT", bufs=1) as hTp,
            tc.tile_pool(name="dout", bufs=3) as dp,
            tc.tile_pool(name="psa", bufs=1, space="PSUM") as psap,
            tc.tile_pool(name="psz", bufs=1, space="PSUM") as pszp,
            tc.tile_pool(name="psd", bufs=3, space="PSUM") as psdp,
        ):
            # ---- queue every load on the sync ring, in issue order ----
            bank_sb = []
            for half in range(2):
                bk = constp.tile([128, BANKW], BF16, tag=f"bank{half}")
                nc.sync.dma_start(
                    bk[:], bank_d[half * 128:(half + 1) * 128, :])
                bank_sb.append(bk)
            hTs = []
            for c in range(NCH):
                hT = hTp.tile([128, S], BF16, tag=f"hT{c}")
                nc.sync.dma_start(hT[:], ht_d[c * 128:(c + 1) * 128, :])
                hTs.append(hT)

            m_sb = [bk[:, 0:R] for bk in bank_sb]
            a_sb = [bk[:, R:R + DIN] for bk in bank_sb]
            b_sb = [bk[:, R + DIN:BANKW] for bk in bank_sb]

            # ---- A_mixT: all 16 chunks into one PSUM bank [128, 256] ----
            # (allocated [128, 512] so the bank is fully padded and the
            # following psz pool stays bank-aligned)
            amix_full = psap.tile([128, 512], F32, tag="am")
            amix_ps = amix_full[:, 0:NCH * R]
            for c in range(NCH):
                csl = slice(c * 128, (c + 1) * 128)
                osl = slice(c * R, (c + 1) * R)
                nc.tensor.matmul(amix_ps[:, osl], a_sb[0][:, csl], m_sb[0],
                                 start=True, stop=False)
                nc.tensor.matmul(amix_ps[:, osl], a_sb[1][:, csl], m_sb[1],
                                 start=False, stop=True)
            amixT = constp.tile([128, NCH * R], BF16, tag="amixT")
            nc.vector.tensor_copy(amixT[:, 0:NCH * R // 2],
                                  amix_ps[:, 0:NCH * R // 2])
            nc.vector.tensor_copy(amixT[:, NCH * R // 2:],
                                  amix_ps[:, NCH * R // 2:])

            # ---- B_mixedT [R, DOUT] = M.T @ B_flat (via mm2 pool) ----
            bmixT = constp.tile([R, DOUT], BF16, tag="bmixT")
            for c4 in range(DOUT // 512):
                sl = slice(c4 * 512, (c4 + 1) * 512)
                pmix = psdp.tile([R, 512], F32, tag="dps", name=f"pmix{c4}")
                nc.tensor.matmul(pmix[:], m_sb[0], b_sb[0][:, sl],
                                 start=True, stop=False)
                nc.tensor.matmul(pmix[:], m_sb[1], b_sb[1][:, sl],
                                 start=False, stop=True)
                nc.scalar.copy(bmixT[:, sl], pmix[:])

            # ---- mm1, c-outer: consume each hT chunk as it arrives ----
            zt_ps = pszp.tile([R, S], F32, tag="zt")
            for c in range(NCH):
                for sc in range(NSC):
                    ssl = slice(sc * 512, (sc + 1) * 512)
                    nc.tensor.matmul(
                        zt_ps[:, ssl], amixT[:, c * R:(c + 1) * R],
                        hTs[c][:, ssl],
                        start=(c == 0), stop=(c == NCH - 1))
            zts = constp.tile([R, S], BF16, tag="zts")
            for sc in range(NSC):
                ssl = slice(sc * 512, (sc + 1) * 512)
                nc.vector.tensor_copy(zts[:, ssl], zt_ps[:, ssl])

            # ---- mm2 + stores: delta tile [128, DOUT] per s-tile ----
            for st in range(S // 128):
                dsb = dp.tile([128, DOUT], BF16, tag="d")
                for oc in range(DOUT // 512):
                    osl = slice(oc * 512, (oc + 1) * 512)
                    dps = psdp.tile([128, 512], F32, tag="dps",
                                    name=f"dps{st}_{oc}")
                    nc.tensor.matmul(
                        dps[:], zts[:, st * 128:(st + 1) * 128],
                        bmixT[:, osl])
                    if oc % 2 == 0:
                        nc.vector.tensor_copy(dsb[:, osl], dps[:])
                    else:
                        nc.scalar.copy(dsb[:, osl], dps[:])
                nc.sync.dma_start(out_d[st * 128:(st + 1) * 128, :], dsb[:])

    nc.compile()
    return nc


def _in_maps(h, alpha, A_bank, B_bank):
    a_flat = A_bank.reshape(KR, DIN).astype(np.float32)
    bt_flat = B_bank.transpose(0, 2, 1).reshape(KR, DOUT).astype(np.float32)
    eye = np.eye(R, dtype=np.float32)
    maps = []
    for b in range(B):
        mix = np.kron(alpha[b].astype(np.float32).reshape(K, 1), eye)
        banks = np.concatenate([mix, a_flat, bt_flat], axis=1)
        hT = np.ascontiguousarray(
            np.asarray(h[b]).T).astype(ml_dtypes.bfloat16)
        maps.append({
            "hbT": hT,
            "banks": np.ascontiguousarray(banks.astype(ml_dtypes.bfloat16)),
        })
    return maps


def _run(inputs, trace=False):
    if "nc" not in _cache:
        _cache["nc"] = _build_nc()
    nc = _cache["nc"]
    maps = _in_maps(inputs["h"], inputs["alpha"], inputs["A_bank"],
                    inputs["B_bank"])
    res = run_bass_kernel_spmd(nc, maps, core_ids=list(range(B)), trace=trace)
    out = np.stack([res.results[b]["delta"] for b in range(B)], axis=0)
    return out.astype(np.float32), res


def kernel(**inputs):
    out, _ = _run(inputs, trace=False)
    return out


# revision 6
# speedup vs baseline: 1.2312x; 1.0098x over previous
"""LoRA basis-bank kernel for 8 TRN2 NeuronCores.

Math (per batch b):
    A_mixed  = sum_k alpha[b,k] * A_bank[k]        # [R, DIN]
    B_mixedT = sum_k alpha[b,k] * B_bank[k].T      # [R, DOUT]
    z        = h[b] @ A_mixed.T                    # [S, R]
    delta[b] = z @ B_mixedT                        # [S, DOUT]

Sharding: data-parallel over batch, 1 batch per core; banks replicated.

Host-side layout prep (no arithmetic): h shard is uploaded transposed
(hT[i, s]) in bf16 so the DIN contraction lands on SBUF partitions with
no on-device transposes; mix/A_bank/B_bank.T are concatenated into one
[K*R, R + DIN + DOUT] bf16 "banks" tensor so they load in 2 big DMAs;
alpha is expanded into a [K*R, R] block-diagonal placement matrix.

Device dataflow per core (memory-roofline streaming schedule):
  - sync ring queue order: banks (2x ~1MB), hT chunks (16x 512KB),
    then the 16 delta stores (ring idle by then; scalar/vector engines
    stay free for PSUM drains).
  - A_mixT [128c, 16] chunks all land in ONE PSUM bank [128, 256],
    two casts to SBUF.
  - B_mixedT [R, DOUT] = M.T @ B_flat via the mm2 PSUM pool.
  - mm1 runs c-outer: each arriving hT chunk [128, S] is immediately
    consumed into one persistent PSUM tile zT [16, 2048] (4 banks),
    so compute streams behind the loads.
  - mm2: per s-tile [128, DOUT] = zT.T @ B_mixedT, 4 matmuls through
    a 3-deep PSUM pool, drained by alternating vector/scalar copies.
  - delta written bf16, upcast to fp32 on host
"""

import ml_dtypes
import numpy as np

import concourse.bacc as bacc
import concourse.bass as bass
import concourse.mybir as mybir
import concourse.tile as tile
from concourse.bass_utils import run_bass_kernel_spmd

B, S, K, R, DIN, DOUT = 8, 2048, 16, 16, 2048, 2048
KR = K * R  # 256
BANKW = R + DIN + DOUT  # 16 + 2048 + 2048 = 4112
F32 = mybir.dt.float32
BF16 = mybir.dt.bfloat16

_cache = {}


def _build_nc():
    nc = bacc.Bacc("TRN2", target_bir_lowering=False)

    ht_d = nc.dram_tensor("hbT", [DIN, S], BF16, kind="ExternalInput")
    bank_d = nc.dram_tensor("banks", [KR, BANKW], BF16, kind="ExternalInput")
    out_d = nc.dram_tensor("delta", [S, DOUT], BF16, kind="ExternalOutput")

    NCH = DIN // 128  # 16 chunks along DIN
    NSC = S // 512    # 4 s-chunks
    with tile.TileContext(nc) as tc:
        with (
            tc.tile_pool(name="const", bufs=1) as constp,
            tc.tile_pool(name="hT", bufs=1) as hTp,
            tc.tile_pool(name="dout", bufs=3) as dp,
            tc.tile_pool(name="psa", bufs=1, space="PSUM") as psap,
            tc.tile_pool(name="psz", bufs=1, space="PSUM") as pszp,
            tc.tile_pool(name="psd", bufs=3, space="PSUM") as psdp,
        ):
            # ---- queue every load on the sync ring, in issue order ----
            bank_sb = []
            for half in range(2):
                bk = constp.tile([128, BANKW], BF16, tag=f"bank{half}")
                nc.sync.dma_start(
                    bk[:], bank_d[half * 128:(half + 1) * 128, :])
                bank_sb.append(bk)
            hTs = []
            for c in range(NCH):
                hT = hTp.tile([128, S], BF16, tag=f"hT{c}")
                nc.sync.dma_start(hT[:], ht_d[c * 128:(c + 1) * 128, :])
                hTs.append(hT)

            m_sb = [bk[:, 0:R] for bk in bank_sb]
            a_sb = [bk[:, R:R + DIN] for bk in bank_sb]
            b_sb = [bk[:, R + DIN:BANKW] for bk in bank_sb]

            # ---- A_mixT: all 16 chunks into one PSUM bank [128, 256] ----
            # (allocated [128, 512] so the bank is fully padded and the
            # following psz pool stays bank-aligned)
            amix_full = psap.tile([128, 512], F32, tag="am")
            amix_ps = amix_full[:, 0:NCH * R]
            for c in range(NCH):
                csl = slice(c * 128, (c + 1) * 128)
                osl = slice(c * R, (c + 1) * R)
                nc.tensor.matmul(amix_ps[:, osl], a_sb[0][:, csl], m_sb[0],
                                 start=True, stop=False)
                nc.tensor.matmul(amix_ps[:, osl], a_sb[1][:, csl], m_sb[1],
                                 start=False, stop=True)
            amixT = constp.tile([128, NCH * R], BF16, tag="amixT")
            nc.vector.tensor_copy(amixT[:, 0:NCH * R // 2],
                                  amix_ps[:, 0:NCH * R // 2])
            nc.vector.tensor_copy(amixT[:, NCH * R // 2:],
                                  amix_ps[:, NCH * R // 2:])

            # ---- B_mixedT [R, DOUT] = M.T @ B_flat (via mm2 pool) ----
            bmixT = constp.tile([R, DOUT], BF16, tag="bmixT")
            for c4 in range(DOUT // 512):
                sl = slice(c4 * 512, (c4 + 1) * 512)
                pmix = psdp.tile([R, 512], F32, tag="dps", name=f"pmix{c4}")
                nc.tensor.matmul(pmix[:], m_sb[0], b_sb[0][:, sl],
                                 start=True, stop=False)
                nc.tensor.matmul(pmix[:], m_sb[1], b_sb[1][:, sl],
                                 start=False, stop=True)
                nc.scalar.copy(bmixT[:, sl], pmix[:])

            # ---- mm1, c-outer: consume each hT chunk as it arrives ----
            zt_ps = pszp.tile([R, S], F32, tag="zt")
            for c in range(NCH):
                for sc in range(NSC):
                    ssl = slice(sc * 512, (sc + 1) * 512)
                    nc.tensor.matmul(
                        zt_ps[:, ssl], amixT[:, c * R:(c + 1) * R],
                        hTs[c][:, ssl],
                        start=(c == 0), stop=(c == NCH - 1))
            # casts split across vector/scalar (different PSUM banks) so
            # the PE-idle window at the phase boundary stays well under
            # the ~3.4us HAM re-throttle window
            zts = constp.tile([R, S], BF16, tag="zts")
            for sc in range(NSC):
                ssl = slice(sc * 512, (sc + 1) * 512)
                if sc % 2 == 0:
                    nc.vector.tensor_copy(zts[:, ssl], zt_ps[:, ssl])
                else:
                    nc.scalar.copy(zts[:, ssl], zt_ps[:, ssl])

            # ---- mm2 + stores: delta tile [128, DOUT] per s-tile ----
            for st in range(S // 128):
                dsb = dp.tile([128, DOUT], BF16, tag="d")
                for oc in range(DOUT // 512):
                    osl = slice(oc * 512, (oc + 1) * 512)
                    dps = psdp.tile([128, 512], F32, tag="dps",
                                    name=f"dps{st}_{oc}")
                    nc.tensor.matmul(
                        dps[:], zts[:, st * 128:(st + 1) * 128],
                        bmixT[:, osl])
                    if oc % 2 == 0:
                        nc.vector.tensor_copy(dsb[:, osl], dps[:])
                    else:
                        nc.scalar.copy(dsb[:, osl], dps[:])
                nc.sync.dma_start(out_d[st * 128:(st + 1) * 128, :], dsb[:])

    nc.compile()
    return nc


def _in_maps(h, alpha, A_bank, B_bank):
    a_flat = A_bank.reshape(KR, DIN).astype(np.float32)
    bt_flat = B_bank.transpose(0, 2, 1).reshape(KR, DOUT).astype(np.float32)
    eye = np.eye(R, dtype=np.float32)
    maps = []
    for b in range(B):
        mix = np.kron(alpha[b].astype(np.float32).reshape(K, 1), eye)
        banks = np.concatenate([mix, a_flat, bt_flat], axis=1)
        hT = np.ascontiguousarray(
            np.asarray(h[b]).T).astype(ml_dtypes.bfloat16)
        maps.append({
            "hbT": hT,
            "banks": np.ascontiguousarray(banks.astype(ml_dtypes.bfloat16)),
        })
    return maps


def _run(inputs, trace=False):
    if "nc" not in _cache:
        _cache["nc"] = _build_nc()
    nc = _cache["nc"]
    maps = _in_maps(inputs["h"], inputs["alpha"], inputs["A_bank"],
                    inputs["B_bank"])
    res = run_bass_kernel_spmd(nc, maps, core_ids=list(range(B)), trace=trace)
    out = np.stack([res.results[b]["delta"] for b in range(B)], axis=0)
    return out.astype(np.float32), res


def kernel(**inputs):
    out, _ = _run(inputs, trace=False)
    return out


# revision 9
# speedup vs baseline: 1.3160x; 1.0689x over previous
"""LoRA basis-bank kernel for 8 TRN2 NeuronCores.

Math (per batch b):
    A_mixed  = sum_k alpha[b,k] * A_bank[k]        # [R, DIN]
    B_mixedT = sum_k alpha[b,k] * B_bank[k].T      # [R, DOUT]
    z        = h[b] @ A_mixed.T                    # [S, R]
    delta[b] = z @ B_mixedT                        # [S, DOUT]

Sharding: data-parallel over batch, 1 batch per core; banks replicated.

Host-side layout prep (no arithmetic): h shard is uploaded transposed
(hT[i, s]) in bf16 so the DIN contraction lands on SBUF partitions with
no on-device transposes; mix/A_bank/B_bank.T are concatenated+folded
into one [128, 2*(R+DIN+DOUT)] bf16 "banks" tensor (one DMA); alpha is
expanded into a [K*R, R] block-diagonal placement matrix.

Device dataflow per core (memory-roofline streaming schedule):
  - sync ring order: banks (1x 2MB), hT chunk-pairs (8x 1MB), then the
    16 delta stores.
  - A_mixT [128c, 16] chunks land in ONE rotating PSUM bank, cast once.
  - B_mixedT replicated at partition strips 0-15 / 32-47 of bmix2.
  - mm1 c-outer: each arriving hT pair is consumed immediately into the
    persistent PSUM tile zT [16, 2048] (4 banks).
  - zT cast to both partition strips of zts2.
  - mm2 2-way row-tiled: s-tile pairs run concurrently in PE row strips
    0-15 / 32-47 (tile_position), so even a HAM-cold PE beats the DMA
    store pace; vector drains strip-0 tiles, scalar strip-1 tiles.
  - delta written bf16, upcast to fp32 on host
"""

import ml_dtypes
import numpy as np

import concourse.bacc as bacc
import concourse.bass as bass
import concourse.mybir as mybir
import concourse.tile as tile
from concourse.bass_utils import run_bass_kernel_spmd

B, S, K, R, DIN, DOUT = 8, 2048, 16, 16, 2048, 2048
KR = K * R  # 256
BANKW = R + DIN + DOUT  # 4112
F32 = mybir.dt.float32
BF16 = mybir.dt.bfloat16

_cache = {}


def _build_nc():
    nc = bacc.Bacc("TRN2", target_bir_lowering=False)

    ht_d = nc.dram_tensor("hbT", [DIN, S], BF16, kind="ExternalInput")
    bank_d = nc.dram_tensor("banks", [128, 2 * BANKW], BF16,
                            kind="ExternalInput")
    out_d = nc.dram_tensor("delta", [S, DOUT], BF16, kind="ExternalOutput")

    NCH = DIN // 128  # 16 chunks along DIN
    NSC = S // 512    # 4 s-chunks
    with tile.TileContext(nc) as tc:
        with (
            tc.tile_pool(name="const", bufs=1) as constp,
            tc.tile_pool(name="hT", bufs=1) as hTp,
            tc.tile_pool(name="dout", bufs=2) as dp,
            tc.tile_pool(name="psz", bufs=1, space="PSUM") as pszp,
            tc.tile_pool(name="psd", bufs=4, space="PSUM") as psdp,
        ):
            # ---- queue every load on the sync ring, in issue order ----
            bank = constp.tile([128, 2 * BANKW], BF16, tag="bank")
            nc.sync.dma_start(bank[:], bank_d[:, :])
            hTs = []
            for i in range(NCH // 2):
                hT = hTp.tile([128, 2, S], BF16, tag=f"hT{i}")
                nc.sync.dma_start(
                    hT[:],
                    ht_d[i * 256:(i + 1) * 256, :].rearrange(
                        "(g p) s -> p g s", g=2))
                hTs.append(hT)

            m_sb = [bank[:, 0:R], bank[:, BANKW:BANKW + R]]
            a_sb = [bank[:, R:R + DIN], bank[:, BANKW + R:BANKW + R + DIN]]
            b_sb = [bank[:, R + DIN:BANKW],
                    bank[:, BANKW + R + DIN:2 * BANKW]]

            # ---- A_mixT: all 16 chunks into one PSUM bank [128, 256] ----
            amix_ps = psdp.tile([128, 512], F32, tag="dps", name="amix_ps")
            for c in range(NCH):
                csl = slice(c * 128, (c + 1) * 128)
                osl = slice(c * R, (c + 1) * R)
                nc.tensor.matmul(amix_ps[:, osl], a_sb[0][:, csl], m_sb[0],
                                 start=True, stop=False)
                nc.tensor.matmul(amix_ps[:, osl], a_sb[1][:, csl], m_sb[1],
                                 start=False, stop=True)
            amixT = constp.tile([128, NCH * R], BF16, tag="amixT")
            nc.vector.tensor_copy(amixT[:, 0:128], amix_ps[:, 0:128])
            nc.vector.tensor_copy(amixT[:, 128:256], amix_ps[:, 128:256])

            # ---- B_mixedT replicated at partition strips 0-15 / 32-47 ----
            bmix2 = constp.tile([48, DOUT], BF16, tag="bmix2")
            for c4 in range(DOUT // 512):
                sl = slice(c4 * 512, (c4 + 1) * 512)
                pmix = psdp.tile([R, 512], F32, tag="dps", name=f"pmix{c4}")
                nc.tensor.matmul(pmix[:], m_sb[0], b_sb[0][:, sl],
                                 start=True, stop=False)
                nc.tensor.matmul(pmix[:], m_sb[1], b_sb[1][:, sl],
                                 start=False, stop=True)
                nc.scalar.copy(bmix2[0:R, sl], pmix[:])
                nc.scalar.copy(bmix2[32:32 + R, sl], pmix[:])

            # ---- mm1, c-outer: consume each hT pair as it arrives ----
            zt_ps = pszp.tile([R, S], F32, tag="zt")
            for c in range(NCH):
                for sc in range(NSC):
                    ssl = slice(sc * 512, (sc + 1) * 512)
                    nc.tensor.matmul(
                        zt_ps[:, ssl], amixT[:, c * R:(c + 1) * R],
                        hTs[c // 2][:, c % 2, ssl],
                        start=(c == 0), stop=(c == NCH - 1))

            # cast z to both strips; split across vector/scalar so the
            # PE-idle window stays far below the HAM re-throttle window
            zts2 = constp.tile([48, S], BF16, tag="zts2")
            for sc in range(NSC):
                ssl = slice(sc * 512, (sc + 1) * 512)
                if sc % 2 == 0:
                    nc.vector.tensor_copy(zts2[0:R, ssl], zt_ps[:, ssl])
                else:
                    nc.scalar.copy(zts2[0:R, ssl], zt_ps[:, ssl])
            for sc in range(NSC):
                ssl = slice(sc * 512, (sc + 1) * 512)
                if sc % 2 == 0:
                    nc.scalar.copy(zts2[32:32 + R, ssl], zt_ps[:, ssl])
                else:
                    nc.vector.tensor_copy(zts2[32:32 + R, ssl], zt_ps[:, ssl])

            # ---- mm2 2-way row-tiled + stores ----
            for pt in range(S // 256):
                st0, st1 = 2 * pt, 2 * pt + 1
                dsb0 = dp.tile([128, DOUT], BF16, tag="d0")
                dsb1 = dp.tile([128, DOUT], BF16, tag="d1")
                for oc in range(DOUT // 512):
                    osl = slice(oc * 512, (oc + 1) * 512)
                    dpsA = psdp.tile([128, 512], F32, tag="dps",
                                     name=f"dpsA{pt}_{oc}")
                    nc.tensor.matmul(
                        dpsA[:], zts2[0:R, st0 * 128:(st0 + 1) * 128],
                        bmix2[0:R, osl], tile_position=(0, 0))
                    dpsB = psdp.tile([128, 512], F32, tag="dps",
                                     name=f"dpsB{pt}_{oc}")
                    nc.tensor.matmul(
                        dpsB[:], zts2[32:32 + R, st1 * 128:(st1 + 1) * 128],
                        bmix2[32:32 + R, osl], tile_position=(32, 0))
                    nc.vector.tensor_copy(dsb0[:, osl], dpsA[:])
                    nc.scalar.copy(dsb1[:, osl], dpsB[:])
                nc.sync.dma_start(out_d[st0 * 128:(st0 + 1) * 128, :],
                                  dsb0[:])
                nc.sync.dma_start(out_d[st1 * 128:(st1 + 1) * 128, :],
                                  dsb1[:])

    nc.compile()
    return nc


def _in_maps(h, alpha, A_bank, B_bank):
    a_flat = A_bank.reshape(KR, DIN).astype(np.float32)
    bt_flat = B_bank.transpose(0, 2, 1).reshape(KR, DOUT).astype(np.float32)
    eye = np.eye(R, dtype=np.float32)
    maps = []
    for b in range(B):
        mix = np.kron(alpha[b].astype(np.float32).reshape(K, 1), eye)
        banks = np.concatenate([mix, a_flat, bt_flat], axis=1)
        banks = banks.reshape(2, 128, BANKW).transpose(1, 0, 2).reshape(
            128, 2 * BANKW)
        hT = np.ascontiguousarray(
            np.asarray(h[b]).T).astype(ml_dtypes.bfloat16)
        maps.append({
            "hbT": hT,
            "banks": np.ascontiguousarray(banks.astype(ml_dtypes.bfloat16)),
        })
    return maps


def _run(inputs, trace=False):
    if "nc" not in _cache:
        _cache["nc"] = _build_nc()
    nc = _cache["nc"]
    maps = _in_maps(inputs["h"], inputs["alpha"], inputs["A_bank"],
                    inputs["B_bank"])
    res = run_bass_kernel_spmd(nc, maps, core_ids=list(range(B)), trace=trace)
    out = np.stack([res.results[b]["delta"] for b in range(B)], axis=0)
    return out.astype(np.float32), res


def kernel(**inputs):
    out, _ = _run(inputs, trace=False)
    return out
